# revision 1
# baseline (speedup 1.0000x reference)
"""Trainium2 Bass kernel for nn_BTRLoss: grayscale morphological opening loss.

Per image: tip = MLP(grid, t) [16x16]; eroded = erosion(image, tip);
recon = dilation(eroded, tip); loss = mean((recon-image)^2) + regularizers.
The tiny tip-MLP and scalar regularizer terms run on the host; the heavy
morphology (2 x 256-tap max-plus ops over 1024x1024) runs on 8 NeuronCores,
one image per core (data-parallel over the batch).

Device strategy per core: the image is a 16x8 grid of 64x128 tiles, one tile
per SBUF partition, stored with its 79x144 halo so both morphology shift
directions are free-dim AP offsets. Each tap k=(u,v) computes
cand = window(u,v) -/+ tip[u,v], then carry = min/max(carry, cand) as a DVE
tensor_tensor (fp16 2x_1P packed mode). The bias op is load-balanced between
the Scalar engine (activation Identity with per-partition bias; takes all
odd-v taps, whose windows are 2-byte-misaligned, plus some even) and the DVE
itself (tensor_scalar, 4x mode, even-v taps only -- measured optimum ~72 DVE
biases per morph). GPSIMD compute is intentionally unused: it shares an SBUF
port with the DVE and halves DVE throughput when active.

The eroded image round-trips through DRAM (zero-padded buffer) to rebuild
halos across partitions; the squared-diff loss reduces on-device to [128,1]
per-partition partials via ACT Square+accum; the host finishes the mean.
Measured ~2.75 ms/core on trn2 (DVE and ACT both >94% busy).
"""
import numpy as np

try:
    import concourse.bass as bass
except ImportError:
    import sys
    for p in ("/opt/trn_rl_repo", "/root/.axon_site/_ro/trn_rl_repo"):
        if p not in sys.path:
            sys.path.insert(0, p)
    import concourse.bass as bass

import concourse.bacc as bacc
import concourse.tile as tile
from concourse import mybir
from concourse.bass_utils import run_bass_kernel_spmd

# ---- problem geometry (hardcoded per spec) ----
B, H, W = 8, 1024, 1024
K = 16
PAD_BEG = 7          # (K-1)//2
TRG, TCG = 16, 8     # partition tile grid: 16 rows x 8 cols = 128 partitions
TH, TW = 64, 128     # per-partition output tile
HR = TH + K - 1      # 79 halo rows
HC = 144             # halo cols (needs 143; padded to even for alignment)
RB = H + K - 1       # 1039 buffer rows
CB = 1042            # buffer cols: image at col 8, reads reach col 1040
IMG_R0, IMG_C0 = PAD_BEG, PAD_BEG + 1  # image origin inside the DRAM buffer

F32 = mybir.dt.float32
F16 = mybir.dt.float16

# tip grid (matches reference)
_x = np.linspace(-K / 2, K / 2, K, dtype=np.float32)
_X, _Y = np.meshgrid(_x, _x, indexing="ij")
XF = _X.reshape(-1)
YF = _Y.reshape(-1)


def _tip_mlp(t, w1, b1, w2, b2, w3, b3):
    inp = np.stack([XF, YF, np.full(K * K, t, np.float32)], axis=-1)
    h = np.tanh((inp @ w1 + b1).astype(np.float32)).astype(np.float32)
    h = np.tanh((h @ w2 + b2).astype(np.float32)).astype(np.float32)
    return ((h @ w3 + b3)[..., 0]).astype(np.float32)  # [256]


def _assign_engines(n_d, n_g):
    """Per-tap bias-engine assignment for one morph op (256 taps).

    Every tap's min/max runs as a DVE tensor_tensor (2x mode); the bias
    (window +- tip[u,v]) runs on one of three engines: 'D' DVE tensor_scalar
    (4x, needs the 4B-aligned even-v windows), 'A' ACT activation-with-bias
    (alignment-free), 'G' GPSIMD tensor_tensor with a broadcast scalar
    operand. Counts: n_d DVE taps (even-v only), n_g GPSIMD taps, rest ACT.
    Tap 0 initializes the carry via DVE tensor_scalar directly.
    """
    eng = {0: 'D'}
    evens = [k for k in range(2, K * K, 2)]
    rest = []
    for i, k in enumerate(evens):
        if (i * n_d) // len(evens) != ((i + 1) * n_d) // len(evens):
            eng[k] = 'D'
        else:
            rest.append(k)
    rest = sorted(rest + list(range(1, K * K, 2)))
    for i, k in enumerate(rest):
        eng[k] = 'G' if (i * n_g) // len(rest) != ((i + 1) * n_g) // len(rest) \
            else 'A'
    return [eng[k] for k in range(K * K)]


def build_nc(dt=F16, n_d=72, n_g=0, cand_bufs=4):
    nc = bacc.Bacc("TRN2", target_bir_lowering=False)
    ahalo = nc.dram_tensor("ahalo", [128, HR * HC], dt, kind="ExternalInput")
    tips = nc.dram_tensor("tips", [1, K * K], F32, kind="ExternalInput")
    ntips = nc.dram_tensor("ntips", [1, K * K], F32, kind="ExternalInput")
    out_ps = nc.dram_tensor("psum", [128, 1], F32, kind="ExternalOutput")

    sub, add = mybir.AluOpType.subtract, mybir.AluOpType.add
    amin, amax, amult = mybir.AluOpType.min, mybir.AluOpType.max, mybir.AluOpType.mult
    COPY = mybir.ActivationFunctionType.Identity
    assign = _assign_engines(n_d, n_g)

    def morph(halo, tips_act, carry, op0, op1, cpool):
        """carry = reduce_{u,v} (window(u,v) op0 tip[u,v]), reduce = op1."""
        ts_init = {sub: nc.vector.tensor_scalar_sub,
                   add: nc.vector.tensor_scalar_add}[op0]
        for kk in range(K * K):
            u, v = kk // K, kk % K
            win = halo[:, u:u + TH, v:v + TW]
            e = assign[kk]
            if kk == 0:
                ts_init(carry, win, tips_sb[:, 0:1])
                continue
            cand = cpool.tile([128, TH, TW], dt, name="cand")
            if e == 'G':
                sc = tips_sb[:, kk:kk + 1]
                bc = bass.AP(sc.tensor, sc.offset, [sc.ap[0], [0, TH], [0, TW]])
                nc.gpsimd.tensor_tensor(out=cand, in0=win, in1=bc, op=op0)
            elif e == 'A':
                nc.scalar.activation(cand, win, COPY,
                                     bias=tips_act[:, kk:kk + 1], scale=1.0)
            else:
                ts_init(cand, win, tips_sb[:, kk:kk + 1])
            nc.vector.tensor_tensor(out=carry, in0=cand, in1=carry, op=op1)

    with tile.TileContext(nc) as tc:
        with tc.tile_pool(name="sb", bufs=1) as sb, \
             tc.tile_pool(name="cand", bufs=cand_bufs) as cpool, \
             tc.tile_pool(name="dram", bufs=1, space="DRAM") as dram:
            tips_sb = sb.tile([128, K * K], F32)
            nc.sync.dma_start(out=tips_sb,
                              in_=bass.AP(tips, 0, [[0, 128], [1, K * K]]))
            negtips_sb = sb.tile([128, K * K], F32)
            nc.sync.dma_start(out=negtips_sb,
                              in_=bass.AP(ntips, 0, [[0, 128], [1, K * K]]))

            hA = sb.tile([128, HR, HC], dt)
            half = 40 * HC
            nc.sync.dma_start(out=hA[:, 0:40, :], in_=ahalo[:, 0:half])
            nc.scalar.dma_start(out=hA[:, 40:HR, :], in_=ahalo[:, half:HR * HC])
            imgT = sb.tile([128, TH, TW], dt)
            nc.sync.dma_start(
                out=imgT,
                in_=bass.AP(ahalo, PAD_BEG * HC + PAD_BEG,
                            [[HR * HC, 128], [HC, TH], [1, TW]]))

            # ---- erosion: ec = min_{u,v} (window - tip[u,v]) ----
            ec = sb.tile([128, TH, TW], dt)
            morph(hA, negtips_sb, ec, sub, amin, cpool)

            # ---- halo exchange via DRAM round-trip (single SWDGE queue) ----
            epad = dram.tile([RB, CB], dt)
            zrow = sb.tile([128, CB], dt)
            nc.gpsimd.memset(zrow, 0.0)
            for i in range(8):
                nc.gpsimd.dma_start(out=epad[i * 128:(i + 1) * 128, :], in_=zrow[:, :])
            nc.gpsimd.dma_start(out=epad[1024:RB, :], in_=zrow[0:RB - 1024, :])
            # interior: eroded tile (tr,tc) -> rows 7+64*tr, cols 8+128*tc
            for tr in range(TRG):
                nc.sync.dma_start(
                    out=bass.AP(epad.tensor,
                                epad.offset + (IMG_R0 + tr * TH) * CB + IMG_C0,
                                [[TW, TCG], [CB, TH], [1, TW]]),
                    in_=ec[tr * TCG:(tr + 1) * TCG, :, :])
            # reload with halos: partition (tr,tc) rows 64*tr.., cols 128*tc+1..
            eA = sb.tile([128, HR, HC], dt)
            for tr in range(TRG):
                nc.scalar.dma_start(
                    out=eA[tr * TCG:(tr + 1) * TCG, :, :],
                    in_=bass.AP(epad.tensor, epad.offset + 1 + tr * TH * CB,
                                [[TW, TCG], [CB, HR], [1, HC]]))

            # ---- dilation: rc = max_{u,v} (window + tip[u,v]) ----
            rc = sb.tile([128, TH, TW], dt)
            morph(eA, tips_sb, rc, add, amax, cpool)

            # ---- loss: psum[p] = sum over tile of (rc - image)^2 ----
            d = sb.tile([128, TH, TW], dt)
            nc.vector.tensor_tensor(out=d, in0=rc, in1=imgT, op=sub)
            ps = sb.tile([128, 1], F32)
            d2 = sb.tile([128, TH, TW], dt)
            nc.scalar.activation(d2, d, mybir.ActivationFunctionType.Square,
                                 accum_out=ps)
            nc.sync.dma_start(out=bass.AP(out_ps, 0, [[1, 128], [1, 1]]), in_=ps)
    nc.compile()
    return nc


_NC_CACHE = {}


def _get_nc():
    if "nc" not in _NC_CACHE:
        _NC_CACHE["nc"] = build_nc()
    return _NC_CACHE["nc"]


def make_halos(img):
    """Host-side gather of the haloed per-partition layout of one image."""
    buf = np.zeros((RB, CB), np.float16)
    buf[IMG_R0:IMG_R0 + H, IMG_C0:IMG_C0 + W] = img
    win = np.lib.stride_tricks.sliding_window_view(buf, (HR, HC))
    a = win[::TH, 1::TW][:TRG, :TCG].reshape(128, HR * HC)
    return np.ascontiguousarray(a)


def _prep_inputs(images, w1, b1, w2, b2, w3, b3, n):
    bhs, in_maps = [], []
    for b in range(B):
        t = float(n * B + b)
        bh = _tip_mlp(t, w1, b1, w2, b2, w3, b3)
        bhs.append(bh)
        in_maps.append({"ahalo": make_halos(images[b]),
                        "tips": bh[None, :].astype(np.float32),
                        "ntips": (-bh)[None, :].astype(np.float32)})
    return bhs, in_maps


def _finish_loss(bhs, results):
    losses = []
    for b in range(B):
        s = float(np.asarray(results[b]["psum"], np.float64).sum())
        recon = s / (H * W)
        bh = bhs[b]
        tip = bh.reshape(K, K)
        boundary = float(np.mean((bh + 100.0) ** 2))
        reg = float(np.sum(bh ** 2))
        cent = float(np.dot(np.abs(bh), XF)) ** 2 + float(np.dot(np.abs(bh), YF)) ** 2
        avg = float(np.mean(bh)) ** 2
        height = float(np.mean(np.maximum(tip, 0.0) ** 2)) + float(np.max(tip)) ** 2
        losses.append(recon + 0.1 * boundary + 1.0 * height
                      + 1e-4 * reg + 0.1 * avg + 1e-3 * cent)
    return np.array(np.mean(np.asarray(losses, np.float64)), dtype=np.float32)


def _run(inputs, trace=False, **kw):
    images = np.asarray(inputs["images"], np.float32)
    args = [np.asarray(inputs[k], np.float32)
            for k in ("w1", "b1", "w2", "b2", "w3", "b3")]
    n = int(np.asarray(inputs["n"]))
    bhs, in_maps = _prep_inputs(images, *args, n)
    res = run_bass_kernel_spmd(_get_nc(), in_maps, core_ids=list(range(B)),
                               trace=trace, **kw)
    return _finish_loss(bhs, res.results), res


def kernel(**inputs) -> np.ndarray:
    loss, _ = _run(inputs)
    return loss



# revision 6
# speedup vs baseline: 16.4272x; 16.4272x over previous
"""Trainium2 Bass kernel for nn_BTRLoss: grayscale morphological opening loss.

Per image: tip = MLP(grid, t) [16x16]; eroded = erosion(image, tip);
recon = dilation(eroded, tip); loss = mean((recon-image)^2) + regularizers.
One image per NeuronCore (data-parallel over the batch of 8).

Algorithm: the two 256-tap max-plus convolutions are computed in the
log/tropical-softmax domain so they become ordinary LINEAR 2D convolutions
that run on the (otherwise idle) 128x128 PE array instead of 512 serial
DVE min/max passes:

    eroded = -max_{u,v}(T - P)  ~=  -(1/b) ln( corr2d(exp(-b P), exp(b T)) )
    recon  =  max_{u,v}(T + E)  ~=   (1/b) ln( corr2d(exp(b E),  exp(b T)) )

with exp(b*eroded) = 1/S available as an exact elementwise reciprocal of the
erosion conv result S (no exp/log needed between the two convs).  b ~ 15 is
chosen per image so every fp32 exponent stays in range; the smooth-max bias
is O(ln(multiplicity)/b) per pixel and measured at ~4e-4 relative error on
the total loss (tolerance 2e-2); host-side prototype proto.py validates.

Device implementation per core:
- layout: rows chunked 10x113 (plus 15 halo rows = 128 partitions per
  chunk); corr2d = 16 PSUM-accumulated bf16 matmuls per [113,512] output
  tile with banded-Toeplitz stationary weights W_v[pin,pout] = K[pin-pout,v]
  (K = exp(b*tip), built on host).  2 convs x 10 chunks x 2 col-halves x 16
  taps = 640 matmuls of 512 moving rows ~ 140us PE.
- erosion tail: DVE reciprocal_approx_fast + bf16 cast; halo rebuild via 3
  band DMAs per chunk into a memset-to-1.0 padded buffer (exp(0)=1 borders
  reproduce the reference's zero padding).
- dilation tail: ACT Ln, DVE subtract of b*I (fp16 upload), ACT Square with
  per-partition accumulate; host sums 128 partials, divides by b^2, adds the
  closed-form regularizer terms (exact, from the host-computed tip MLP).
"""
import numpy as np

try:
    import concourse.bass as bass
except ImportError:
    import sys
    for p in ("/opt/trn_rl_repo", "/root/.axon_site/_ro/trn_rl_repo"):
        if p not in sys.path:
            sys.path.insert(0, p)
    import concourse.bass as bass

import ml_dtypes
import concourse.bacc as bacc
import concourse.tile as tile
from concourse import mybir
from concourse.bass_utils import run_bass_kernel_spmd

# ---- problem geometry (hardcoded per spec) ----
B, H, W = 8, 1024, 1024
K = 16
PB = 7                   # (K-1)//2 pad before
CH = 113                 # output rows per chunk (128 - 15 halo)
NCH = 10                 # ceil(1024/113)
XW = 1040                # padded-column buffer width (needs 1039)
HB = 512                 # column half width (PSUM bank = 512 fp32)

F32 = mybir.dt.float32
F16 = mybir.dt.float16
BF16 = mybir.dt.bfloat16

# tip grid (matches reference)
_x = np.linspace(-K / 2, K / 2, K, dtype=np.float32)
_X, _Y = np.meshgrid(_x, _x, indexing="ij")
XF = _X.reshape(-1)
YF = _Y.reshape(-1)


def _tip_mlp(t, w1, b1, w2, b2, w3, b3):
    inp = np.stack([XF, YF, np.full(K * K, t, np.float32)], axis=-1)
    h = np.tanh((inp @ w1 + b1).astype(np.float32)).astype(np.float32)
    h = np.tanh((h @ w2 + b2).astype(np.float32)).astype(np.float32)
    return ((h @ w3 + b3)[..., 0]).astype(np.float32)  # [256]


def build_nc():
    nc = bacc.Bacc("TRN2", target_bir_lowering=False)
    xe_d = nc.dram_tensor("xe", [128, NCH * XW], BF16, kind="ExternalInput")
    io_d = nc.dram_tensor("iout", [128, NCH * 1024], F16, kind="ExternalInput")
    w_d = nc.dram_tensor("wmat", [128, K * 128], BF16, kind="ExternalInput")
    out_d = nc.dram_tensor("psum", [128, 1], F32, kind="ExternalOutput")

    LN = mybir.ActivationFunctionType.Ln
    SQ = mybir.ActivationFunctionType.Square
    sub = mybir.AluOpType.subtract

    with tile.TileContext(nc) as tc:
        with tc.tile_pool(name="sb", bufs=1) as sb, \
             tc.tile_pool(name="pp", bufs=4, space="PSUM") as pp, \
             tc.tile_pool(name="sc", bufs=2) as scp, \
             tc.tile_pool(name="ln", bufs=2) as lnp, \
             tc.tile_pool(name="df", bufs=2) as dfp:
            WT = sb.tile([128, K, 128], BF16)
            XeT = sb.tile([128, NCH, XW], BF16)
            YiT = sb.tile([128, NCH, 1024], BF16)   # eroded exp (interior)
            YeL = sb.tile([128, NCH, XW], BF16)     # dilation input w/ halos
            IoT = sb.tile([128, NCH, 1024], F16)    # beta * image
            ps = sb.tile([128, 2 * NCH], F32)
            fin = sb.tile([128, 1], F32)

            nc.vector.memset(ps, 0.0)
            nc.vector.memset(YeL, 1.0)              # exp(0): zero-pad borders

            # --- input DMAs; W first (gates all matmuls), Xe chunk-wise ---
            nc.sync.dma_start(out=WT, in_=w_d[:, :])
            qs = (nc.gpsimd, nc.scalar, nc.sync)
            for c in range(NCH):
                qs[c % 2].dma_start(out=XeT[:, c, :],
                                    in_=xe_d[:, c * XW:(c + 1) * XW])
            for c in range(NCH):
                qs[c % 3].dma_start(out=IoT[:, c, :],
                                    in_=io_d[:, c * 1024:(c + 1) * 1024])

            # --- erosion: S = corr2d(Xe, K); Yi = bf16(1/S) ---
            for c in range(NCH):
                nv = min(CH, H - CH * c)
                for h in range(2):
                    pt = pp.tile([128, HB], F32, name="pe")
                    for v in range(K):
                        o = HB * h + v
                        nc.tensor.matmul(pt, WT[:, v, :], XeT[:, c, o:o + HB],
                                         start=(v == 0), stop=(v == K - 1))
                    rc = scp.tile([128, HB], F32, name="rc")
                    nc.vector.reciprocal_approx_fast(rc[0:CH, :], pt[0:CH, :])
                    nc.vector.tensor_scalar_add(
                        YiT[0:CH, c, HB * h:HB * (h + 1)], rc[0:CH, :], 0.0)
                # halo band DMAs into YeL (partition-shifted; DMA only)
                dq = (nc.gpsimd, nc.sync)[c % 2]
                dq.dma_start(out=YeL[PB:PB + nv, c, PB:PB + 1024],
                             in_=YiT[0:nv, c, :])
                if c + 1 < NCH:
                    dq.dma_start(out=YeL[0:PB, c + 1, PB:PB + 1024],
                                 in_=YiT[CH - PB:CH, c, :])
                if c >= 1:
                    nb = min(K - PB - 1, H - CH * c)
                    dq.dma_start(out=YeL[CH + PB:CH + PB + nb, c - 1,
                                         PB:PB + 1024],
                                 in_=YiT[0:nb, c, :])

            # --- dilation: S2 = corr2d(YeL, K); loss partials ---
            for c in range(NCH):
                nv = min(CH, H - CH * c)
                for h in range(2):
                    pt2 = pp.tile([128, HB], F32, name="pd")
                    for v in range(K):
                        o = HB * h + v
                        nc.tensor.matmul(pt2, WT[:, v, :], YeL[:, c, o:o + HB],
                                         start=(v == 0), stop=(v == K - 1))
                    lnT = lnp.tile([128, HB], F32, name="ln")
                    nc.scalar.activation(lnT[0:nv, :], pt2[0:nv, :], LN)
                    dT = dfp.tile([128, HB], F32, name="df")
                    nc.vector.tensor_tensor(
                        out=dT[0:nv, :], in0=lnT[0:nv, :],
                        in1=IoT[0:nv, c, HB * h:HB * (h + 1)], op=sub)
                    sqT = dfp.tile([128, HB], F32, name="sq")
                    col = 2 * c + h
                    nc.scalar.activation(sqT[0:nv, :], dT[0:nv, :], SQ,
                                         accum_out=ps[0:nv, col:col + 1])

            nc.vector.tensor_reduce(fin, ps, mybir.AxisListType.X,
                                    mybir.AluOpType.add)
            nc.sync.dma_start(out=out_d[:, :], in_=fin)
    nc.compile()
    return nc


_NC_CACHE = {}


def _get_nc():
    if "nc" not in _NC_CACHE:
        _NC_CACHE["nc"] = build_nc()
    return _NC_CACHE["nc"]


def _choose_beta(img, bh):
    t_max = float(bh.max())
    p_min = float(img.min())
    p_max = float(img.max())
    caps = [15.0]
    if t_max - p_min > 0:
        caps.append(79.0 / (t_max - p_min))   # erosion conv overflow
    if -p_min > 0:
        caps.append(82.0 / (-p_min))          # dilation conv underflow
    if p_max > 0:
        caps.append(79.0 / p_max)             # dilation conv overflow
    return min(caps)


def _prep_image(img, bh, beta):
    """Build the three per-core upload tensors for one image."""
    T = bh.reshape(K, K)
    Khat = np.exp(beta * T).astype(np.float32)            # [16,16]

    # banded-Toeplitz weights W[p, v, q] = Khat[p-q, v] (0 <= p-q < 16)
    p = np.arange(128)[:, None]
    q = np.arange(128)[None, :]
    d = p - q
    mask = (d >= 0) & (d < K)
    Wf = np.zeros((128, 128, K), np.float32)
    Wf[mask] = Khat[d[mask], :]
    wmat = np.ascontiguousarray(
        Wf.transpose(0, 2, 1)).reshape(128, K * 128).astype(ml_dtypes.bfloat16)

    # padded exp image, chunked with 15-row overlap: [128, 10, 1040]
    full = np.zeros((CH * (NCH - 1) + 128, XW), np.float32)
    full[PB:PB + H, PB:PB + W] = img
    Xf = np.exp(-beta * full)
    idx = (CH * np.arange(NCH))[:, None] + np.arange(128)[None, :]
    xe = np.ascontiguousarray(
        Xf[idx].transpose(1, 0, 2)).reshape(128, NCH * XW).astype(
            ml_dtypes.bfloat16)

    # beta*image in output-chunk layout: [128, 10, 1024] fp16
    rows = np.zeros((CH * (NCH - 1) + 128, W), np.float32)
    rows[0:H] = beta * img
    iout = np.ascontiguousarray(
        rows[idx].transpose(1, 0, 2)).reshape(128, NCH * 1024).astype(
            np.float16)
    return {"xe": xe, "iout": iout, "wmat": wmat}


def _prep_inputs(images, w1, b1, w2, b2, w3, b3, n):
    metas, in_maps = [], []
    for b in range(B):
        t = float(n * B + b)
        bh = _tip_mlp(t, w1, b1, w2, b2, w3, b3)
        img = images[b]
        beta = _choose_beta(img, bh)
        metas.append((bh, beta))
        in_maps.append(_prep_image(img, bh, beta))
    return metas, in_maps


def _finish_loss(metas, results):
    losses = []
    for b in range(B):
        bh, beta = metas[b]
        s = float(np.asarray(results[b]["psum"], np.float64).sum())
        recon = s / (beta * beta) / (H * W)
        tip = bh.reshape(K, K)
        boundary = float(np.mean((bh + 100.0) ** 2))
        reg = float(np.sum(bh ** 2))
        cent = float(np.dot(np.abs(bh), XF)) ** 2 + \
            float(np.dot(np.abs(bh), YF)) ** 2
        avg = float(np.mean(bh)) ** 2
        height = float(np.mean(np.maximum(tip, 0.0) ** 2)) + \
            float(np.max(tip)) ** 2
        losses.append(recon + 0.1 * boundary + 1.0 * height
                      + 1e-4 * reg + 0.1 * avg + 1e-3 * cent)
    return np.array(np.mean(np.asarray(losses, np.float64)), dtype=np.float32)


def _run(inputs, trace=False, **kw):
    images = np.asarray(inputs["images"], np.float32)
    args = [np.asarray(inputs[k], np.float32)
            for k in ("w1", "b1", "w2", "b2", "w3", "b3")]
    n = int(np.asarray(inputs["n"]))
    metas, in_maps = _prep_inputs(images, *args, n)
    res = run_bass_kernel_spmd(_get_nc(), in_maps, core_ids=list(range(B)),
                               trace=trace, **kw)
    return _finish_loss(metas, res.results), res


def kernel(**inputs) -> np.ndarray:
    loss, _ = _run(inputs)
    return loss
